# revision 2
# baseline (speedup 1.0000x reference)
"""
w4a8 fake-quant linear for Trainium2, 8-core SPMD.

  y[b,s,o] = x_dq[b,s,:] . w_dq[o,:]
    x_dq: per-token int8 fake quant-dequant of x
    w_dq: per-channel-group dequant of int4 weights

Sharding: tokens (B*S = 16384) split across the 8 cores; each core computes
its [2048, 2048] output slice against the full weight matrix (compute-bound;
weight/output sharding would force every core to re-read all of x and be
HBM-bound instead).

Host prep: weights are dequantized to bf16 and pre-transposed to [I, O]
(one-time O(N^2) repack; values are int4-grid * group scale, bf16 rounding
is ~2^-9 relative on the weight side only).

Device math: per-token quant produces n = clip(round(x/s)+zp) - zp, which is
an *integer* in [-255, 255] -- exactly representable in bf16.  The matmul
accumulates in fp32 PSUM, and the per-token scale s is applied on PSUM
eviction.  round() is jnp-compatible RNE via the magic-number trick.

Engine layout (v2):
  PE      1024 matmuls only (16 token tiles x 16 kk x 4 psum banks), kept
          back-to-back; stationary operand is the transposed activation
          chunk, weights stream 512-wide.
  DVE     per-token stats (max/min/scale) + quant (magic-round, clip).
  ACT     PSUM eviction (copy*scale, per-partition scale AP) and the
          n->nt DMA-transpose issue (its own HWDGE ring, so transposes
          never queue behind the weight stream).
  SP      weight stream (6 ascending chunks; few DMAs -> no sem-lane
          pacing stalls) then y stores.
  GPSIMD  x tile loads (SWDGE).
"""

import os

import numpy as np
import ml_dtypes

import concourse.bass as bass
import concourse.mybir as mybir
import concourse.tile as tile
from concourse.bass_utils import run_bass_kernel_spmd


def _legalize_waits(nc):
    """Split multi-wait instructions for this walrus build.

    The neuronxcc walrus here supports exactly ONE sync wait per TPB
    instruction (setupSyncWait raises "Too many sync wait commands"
    otherwise).  Tile emits up to ~3 waits per instruction.  Every engine
    executes its instruction stream in order, so hoisting the extra waits
    into standalone EVENT_SEMAPHORE instructions placed immediately before
    the instruction (on the same engine) is semantically identical.
    """
    import bass_rust

    fn = nc.m.functions[0]
    ctr = 0
    new_blocks = []
    for b in fn.blocks:
        out = []
        for i in b.instructions:
            si = i.sync_info
            if si is not None and len(si.on_wait) > 1:
                waits = list(si.on_wait)
                # For DMAs keep the own-lane (ring pacing) wait attached if
                # present; otherwise keep the last one.  All other waits
                # become standalone event-sem stalls just before it.
                own = {u.ant_name for u in si.on_update}
                keep_idx = len(waits) - 1
                for k, w in enumerate(waits):
                    if w.ant_name in own:
                        keep_idx = k
                        break
                for k, w in enumerate(waits):
                    if k == keep_idx:
                        continue
                    ctr += 1
                    es = mybir.InstEventSemaphore(name=f"I-eswait{ctr}")
                    es.engine = i.engine
                    es.sync_info = mybir.SyncInfo(on_wait=[w], on_update=[])
                    out.append(es)
                si.on_wait = [waits[keep_idx]]
            out.append(i)
        new_blocks.append(bass_rust.BasicBlock(name=b.name, instructions=out))
    fn.blocks = new_blocks

NCORES = 8
B, S, I, O = 4, 4096, 2048, 2048
GROUP = 32
TOK = B * S            # 16384 tokens
TPC = TOK // NCORES    # 2048 tokens per core
P = 128
TT = TPC // P          # 16 token tiles per core
KK = I // P            # 16 contraction chunks
NBANK = 512            # fp32 PSUM bank width
NJ = O // NBANK        # 4 psum banks per token tile

MAGIC = 12582912.0     # 1.5 * 2**23: RNE round for |v| < 2**22
EPS = float(np.finfo(np.float32).eps)

# weight stream chunk boundaries (kk units): small leading chunks so the
# first matmuls start as soon as possible, large trailing ones to keep the
# DMA count low (the Tile DMAHW sem lanes are a scarce pacing resource).
W_CHUNKS = [(0, 1), (1, 2), (2, 4), (4, 8), (8, 12), (12, 16)]

_cached_nc = None
last_results = None    # for test harness introspection (exec_time_ns etc.)


def _build_nc():
    nc = bass.Bass()
    f32 = mybir.dt.float32
    bf16 = mybir.dt.bfloat16
    X = mybir.AxisListType.X
    A = mybir.AluOpType

    # Per-token-tile DRAM tensors: Tile tracks DRAM conflicts at tensor
    # granularity, so a single x/y tensor would chain every load/store DMA
    # into a WAW/WAR sequence.
    xs = [
        nc.declare_dram_parameter(f"x{t:02d}", [P, I], f32, isOutput=False)
        for t in range(TT)
    ]
    wt = nc.declare_dram_parameter("wt", [I, O], bf16, isOutput=False)
    ys = [
        nc.declare_dram_parameter(f"y{t:02d}", [P, O], f32, isOutput=True)
        for t in range(TT)
    ]

    with tile.TileContext(nc) as tc:
        with (
            tc.tile_pool(name="wpool", bufs=1) as wpool,
            tc.tile_pool(name="xpool", bufs=3) as xpool,
            tc.tile_pool(name="npool", bufs=2) as npool,
            tc.tile_pool(name="ntpool", bufs=3) as ntpool,
            tc.tile_pool(name="ypool", bufs=2) as ypool,
            tc.tile_pool(name="small", bufs=6) as small,
            tc.tile_pool(name="psum_y", bufs=2, space="PSUM") as psum_y,
        ):
            # x tile 0 rides the ACT HWDGE ring (idle at t=0), in two
            # column halves so the stats reduces can start on the first
            # half while the second is still in flight.
            x_tiles = {}
            x_t0 = xpool.tile([P, I], f32)
            nc.scalar.dma_start(out=x_t0[:, 0:I // 2], in_=xs[0][:, 0:I // 2])
            nc.scalar.dma_start(out=x_t0[:, I // 2:I], in_=xs[0][:, I // 2:I])
            x_tiles[0] = x_t0
            # next two tiles prefetch on gpsimd immediately
            for t in (1, 2):
                x_t = xpool.tile([P, I], f32)
                nc.gpsimd.dma_start(out=x_t, in_=xs[t][:, :])
                x_tiles[t] = x_t

            # resident transposed weights: wt_sb[p, kk, o] = w_dq[o, kk*128+p]
            # (host pre-transposes), streamed in ascending-kk chunks on the
            # SP HWDGE ring.
            wt_sb = wpool.tile([P, KK, O], bf16)
            wt_r = wt.rearrange("(kk p) o -> p kk o", p=P)
            for a, b in W_CHUNKS:
                nc.sync.dma_start(out=wt_sb[:, a:b, :], in_=wt_r[:, a:b, :])

            def quant_stats(x_t, t):
                """Per-token scale pipeline; returns (s, inv, hi)."""
                mx = small.tile([P, 1], f32, tag="mx")
                mn = small.tile([P, 1], f32, tag="mn")
                if t == 0:
                    # split reduces so they start on the first half-DMA
                    mx2 = small.tile([P, 2], f32, tag="mx2")
                    mn2 = small.tile([P, 2], f32, tag="mn2")
                    nc.vector.tensor_reduce(mx2[:, 0:1], x_t[:, 0:I // 2], X, A.max)
                    nc.vector.tensor_reduce(mn2[:, 0:1], x_t[:, 0:I // 2], X, A.min)
                    nc.vector.tensor_reduce(mx2[:, 1:2], x_t[:, I // 2:I], X, A.max)
                    nc.vector.tensor_reduce(mn2[:, 1:2], x_t[:, I // 2:I], X, A.min)
                    nc.vector.tensor_tensor(mx, mx2[:, 0:1], mx2[:, 1:2], A.max)
                    nc.vector.tensor_tensor(mn, mn2[:, 0:1], mn2[:, 1:2], A.min)
                else:
                    nc.vector.tensor_reduce(mx, x_t, X, A.max)
                    nc.vector.tensor_reduce(mn, x_t, X, A.min)
                nc.vector.tensor_scalar(mx, mx, 0.0, None, A.max)
                nc.vector.tensor_scalar(mn, mn, 0.0, None, A.min)
                # s = max((mx - mn)/255, eps); inv = 1/s
                # (DVE has no divide ALU op; *1/255 differs by <=1 ulp)
                s = small.tile([P, 1], f32, tag="s")
                nc.vector.tensor_tensor(s, mx, mn, A.subtract)
                nc.vector.tensor_scalar(s, s, 1.0 / 255.0, EPS, A.mult, A.max)
                inv = small.tile([P, 1], f32, tag="inv")
                nc.vector.reciprocal(inv, s)
                # hi = 127 - zp = 255 + round(mn * inv)
                hi = small.tile([P, 1], f32, tag="hi")
                nc.vector.tensor_tensor(hi, mn, inv, A.mult)
                nc.vector.tensor_scalar(hi, hi, MAGIC, None, A.add)
                nc.vector.tensor_scalar(hi, hi, MAGIC, 255.0, A.subtract, A.add)
                return s, inv, hi

            def quant_and_transpose(x_t, inv, hi, t):
                """n = min(round(x*inv), hi) as bf16, then DMA-xbar
                transpose to nt[p, kk, tok].  Tile 0 is chunked so the
                first nt columns exist as early as possible."""
                q = npool.tile([P, I], f32, tag="q")
                n_bf = npool.tile([P, I], bf16, tag="n")
                nt = ntpool.tile([P, KK, P], bf16)
                nchunk = 4 if t == 0 else 1
                cw = I // nchunk
                ckk = KK // nchunk
                for c in range(nchunk):
                    sl = slice(c * cw, (c + 1) * cw)
                    nc.vector.tensor_scalar(q[:, sl], x_t[:, sl], inv, MAGIC,
                                            A.mult, A.add)
                    nc.vector.tensor_scalar(n_bf[:, sl], q[:, sl], MAGIC, hi,
                                            A.subtract, A.min)
                    nc.scalar.dma_start_transpose(
                        nt[:, c * ckk:(c + 1) * ckk, :], n_bf[:, sl])
                return nt

            # software pipeline: stats/quant/transpose run one tile ahead
            # of the matmuls.
            stats = {0: quant_stats(x_tiles[0], 0)}
            nts = {0: quant_and_transpose(x_tiles[0], stats[0][1], stats[0][2], 0)}

            for tt in range(TT):
                # prefetch x two tiles ahead (gpsimd SWDGE)
                if tt + 3 <= TT - 1:
                    x_t = xpool.tile([P, I], f32)
                    nc.gpsimd.dma_start(out=x_t, in_=xs[tt + 3][:, :])
                    x_tiles[tt + 3] = x_t
                # produce nt for the next tile before this tile's eviction
                # ops so the ACT ring isn't blocked behind them.
                if tt + 1 < TT:
                    stats[tt + 1] = quant_stats(x_tiles[tt + 1], tt + 1)
                    nts[tt + 1] = quant_and_transpose(
                        x_tiles[tt + 1], stats[tt + 1][1], stats[tt + 1][2],
                        tt + 1)
                    del x_tiles[tt + 1]

                nt = nts.pop(tt)
                s = stats.pop(tt)[0]
                ypsum = psum_y.tile([P, O], f32)
                y_sb = ypool.tile([P, O], f32)
                for kk in range(KK):
                    for j in range(NJ):
                        nc.tensor.matmul(
                            ypsum[:, j * NBANK:(j + 1) * NBANK],
                            lhsT=nt[:, kk, :],
                            rhs=wt_sb[:, kk, j * NBANK:(j + 1) * NBANK],
                            start=(kk == 0),
                            stop=(kk == KK - 1),
                        )
                # evict per psum bank on ACT (copy * per-token scale), then
                # store on the SP ring (idle once the weights are in).
                for j in range(NJ):
                    sl = slice(j * NBANK, (j + 1) * NBANK)
                    nc.scalar.mul(y_sb[:, sl], ypsum[:, sl], s)
                    nc.sync.dma_start(out=ys[tt][:, sl], in_=y_sb[:, sl])

    _legalize_waits(nc)
    return nc


def kernel(x, w_q, w_scales, w_zeros):
    global _cached_nc, last_results
    if _cached_nc is None:
        _cached_nc = _build_nc()
    nc = _cached_nc

    x2 = np.ascontiguousarray(np.asarray(x, dtype=np.float32).reshape(TOK, I))
    s_e = np.repeat(np.asarray(w_scales, dtype=np.float32), GROUP, axis=1)
    z_e = np.repeat(np.asarray(w_zeros, dtype=np.float32), GROUP, axis=1)
    w_dq = (np.asarray(w_q).astype(np.float32) - z_e) * s_e
    wt = np.ascontiguousarray(w_dq.T).astype(ml_dtypes.bfloat16)

    in_maps = []
    for c in range(NCORES):
        m = {"wt": wt}
        for t in range(TT):
            base = c * TPC + t * P
            m[f"x{t:02d}"] = x2[base:base + P]
        in_maps.append(m)
    trace = os.environ.get("BASS_KERNEL_TRACE") == "1"
    res = run_bass_kernel_spmd(nc, in_maps, list(range(NCORES)), trace=trace)
    last_results = res
    out = np.concatenate(
        [res.results[c][f"y{t:02d}"] for c in range(NCORES) for t in range(TT)],
        axis=0,
    )
    return np.ascontiguousarray(out.reshape(B, S, O).astype(np.float32))


# revision 7
# speedup vs baseline: 1.0878x; 1.0878x over previous
"""
w4a8 fake-quant linear for Trainium2, 8-core SPMD.

  y[b,s,o] = x_dq[b,s,:] . w_dq[o,:]
    x_dq: per-token int8 fake quant-dequant of x
    w_dq: per-channel-group dequant of int4 weights

Sharding: tokens (B*S = 16384) split across the 8 cores; each core computes
its [2048, 2048] output slice against the full weight matrix (compute-bound;
weight/output sharding would force every core to re-read all of x and be
HBM-bound instead).

Host prep: weights are dequantized to bf16 and pre-transposed to [I, O]
(one-time O(N^2) repack; values are int4-grid * group scale, bf16 rounding
is ~2^-9 relative on the weight side only).

Device math: per-token quant produces n = clip(round(x/s)+zp) - zp, which is
an *integer* in [-255, 255] -- exactly representable in bf16.  The matmul
accumulates in fp32 PSUM, and the per-token scale s is applied on PSUM
eviction.  round() is jnp-compatible RNE via the magic-number trick.

Engine layout (v3):
  PE      warmup matmuls (HAM un-throttle) then 1024 real matmuls,
          kk-outer / psum-bank-inner, back-to-back.
  DVE     per-token stats + quant + PSUM eviction (copy*scale).
  ACT     n->nt DMA-transpose issue only (own HWDGE ring, so transposes
          never queue behind the weight stream).
  SP      x0 (4 column chunks, ahead of the weights), weight stream in
          ascending-kk chunks with x1/x2 slotted in, then y stores.
  GPSIMD  x3.. tile loads (SWDGE), paced by the x pool.
"""

import os

import numpy as np
import ml_dtypes

import concourse.bass as bass
import concourse.mybir as mybir
import concourse.tile as tile
from concourse.bass_utils import run_bass_kernel_spmd


def _legalize_waits(nc):
    """Split multi-wait instructions for this walrus build.

    The neuronxcc walrus here supports exactly ONE sync wait per TPB
    instruction (setupSyncWait raises "Too many sync wait commands"
    otherwise).  Tile emits up to ~3 waits per instruction.  Every engine
    executes its instruction stream in order, so hoisting the extra waits
    into standalone EVENT_SEMAPHORE instructions placed immediately before
    the instruction (on the same engine) is semantically identical.
    """
    import bass_rust

    fn = nc.m.functions[0]
    ctr = 0
    new_blocks = []
    for b in fn.blocks:
        out = []
        for i in b.instructions:
            si = i.sync_info
            if si is not None and len(si.on_wait) > 1:
                waits = list(si.on_wait)
                own = {u.ant_name for u in si.on_update}
                keep_idx = len(waits) - 1
                for k, w in enumerate(waits):
                    if w.ant_name in own:
                        keep_idx = k
                        break
                for k, w in enumerate(waits):
                    if k == keep_idx:
                        continue
                    ctr += 1
                    es = mybir.InstEventSemaphore(name=f"I-eswait{ctr}")
                    es.engine = i.engine
                    es.sync_info = mybir.SyncInfo(on_wait=[w], on_update=[])
                    out.append(es)
                si.on_wait = [waits[keep_idx]]
            out.append(i)
        new_blocks.append(bass_rust.BasicBlock(name=b.name, instructions=out))
    fn.blocks = new_blocks

NCORES = 8
B, S, I, O = 4, 4096, 2048, 2048
GROUP = 32
TOK = B * S            # 16384 tokens
TPC = TOK // NCORES    # 2048 tokens per core
P = 128
TT = TPC // P          # 16 token tiles per core
KK = I // P            # 16 contraction chunks
NBANK = 512            # fp32 PSUM bank width
NJ = O // NBANK        # 4 psum banks per token tile

MAGIC = 12582912.0     # 1.5 * 2**23: RNE round for |v| < 2**22
EPS = float(np.finfo(np.float32).eps)
WARM_N = 44            # dummy matmuls to lift the PE HAM throttle early

_cached_nc = None
last_results = None    # for test harness introspection (exec_time_ns etc.)


def _build_nc():
    nc = bass.Bass()
    f32 = mybir.dt.float32
    bf16 = mybir.dt.bfloat16
    X = mybir.AxisListType
    A = mybir.AluOpType

    xs = [
        nc.declare_dram_parameter(f"x{t:02d}", [P, I], f32, isOutput=False)
        for t in range(TT)
    ]
    wt = nc.declare_dram_parameter("wt", [I, O], bf16, isOutput=False)
    ys = [
        nc.declare_dram_parameter(f"y{t:02d}", [P, O], f32, isOutput=True)
        for t in range(TT)
    ]

    with tile.TileContext(nc) as tc:
        with (
            tc.tile_pool(name="wpool", bufs=1) as wpool,
            tc.tile_pool(name="consts", bufs=1) as consts,
            tc.tile_pool(name="xpool", bufs=3) as xpool,
            tc.tile_pool(name="qpool", bufs=1) as qpool,
            tc.tile_pool(name="npool", bufs=2) as npool,
            tc.tile_pool(name="ntpool", bufs=3) as ntpool,
            tc.tile_pool(name="ypool", bufs=2) as ypool,
            tc.tile_pool(name="small", bufs=8) as small,
            tc.tile_pool(name="psum_y", bufs=2, space="PSUM") as psum_y,
        ):
            # --- PE warm-up: junk matmuls so the HAM clock-gate opens
            # (K=8/8) before the real stream begins.  Uses a memset tile
            # and the first psum pool slot (released before tile 1).
            junk = consts.tile([P, 2 * P], bf16)
            nc.gpsimd.memset(junk, 0.0)
            wpsum = psum_y.tile([P, O], f32, tag="yp")
            for _ in range(WARM_N):
                nc.tensor.matmul(wpsum[:, 0:P], lhsT=junk[:, 0:P],
                                 rhs=junk[:, P:2 * P], start=True, stop=True)

            # --- x tile 0: four column chunks at the HEAD of the SP ring,
            # ahead of the weight stream, so the tile-0 stats/quant chain
            # starts as early as possible.
            x_tiles = {}
            x_t0 = xpool.tile([P, I], f32)
            C4 = I // 4
            for c in range(4):
                nc.sync.dma_start(out=x_t0[:, c * C4:(c + 1) * C4],
                                  in_=xs[0][:, c * C4:(c + 1) * C4])
            x_tiles[0] = x_t0

            # --- weight stream: ascending kk chunks; x1/x2 slotted into
            # the same FIFO ring at the points where they're needed.
            wt_sb = wpool.tile([P, KK, O], bf16)
            wt_r = wt.rearrange("(kk p) o -> p kk o", p=P)
            for a, b in [(0, 1), (1, 2), (2, 4)]:
                nc.sync.dma_start(out=wt_sb[:, a:b, :], in_=wt_r[:, a:b, :])
            x_t1 = xpool.tile([P, I], f32)
            nc.sync.dma_start(out=x_t1, in_=xs[1][:, :])
            x_tiles[1] = x_t1
            for a, b in [(4, 8), (8, 12), (12, 16)]:
                nc.sync.dma_start(out=wt_sb[:, a:b, :], in_=wt_r[:, a:b, :])
            x_t2 = xpool.tile([P, I], f32)
            nc.sync.dma_start(out=x_t2, in_=xs[2][:, :])
            x_tiles[2] = x_t2

            def quant_stats(x_t, t):
                """Per-token scale pipeline; returns (s, inv, hi)."""
                mx = small.tile([P, 1], f32, tag="mx")
                mn = small.tile([P, 1], f32, tag="mn")
                if t == 0:
                    # 4 partial reduces chained to the 4 x0 chunk DMAs
                    mx4 = small.tile([P, 4], f32, tag="mx4")
                    mn4 = small.tile([P, 4], f32, tag="mn4")
                    for c in range(4):
                        sl = slice(c * C4, (c + 1) * C4)
                        nc.vector.tensor_reduce(mx4[:, c:c + 1], x_t[:, sl],
                                                X.X, A.max)
                        nc.vector.tensor_reduce(mn4[:, c:c + 1], x_t[:, sl],
                                                X.X, A.min)
                    nc.vector.tensor_reduce(mx, mx4, X.X, A.max)
                    nc.vector.tensor_reduce(mn, mn4, X.X, A.min)
                else:
                    nc.vector.tensor_reduce(mx, x_t, X.X, A.max)
                    nc.vector.tensor_reduce(mn, x_t, X.X, A.min)
                nc.vector.tensor_scalar(mx, mx, 0.0, None, A.max)
                nc.vector.tensor_scalar(mn, mn, 0.0, None, A.min)
                # s = max((mx - mn)/255, eps); inv = 1/s
                s = small.tile([P, 1], f32, tag="s")
                nc.vector.tensor_tensor(s, mx, mn, A.subtract)
                nc.vector.tensor_scalar(s, s, 1.0 / 255.0, EPS, A.mult, A.max)
                inv = small.tile([P, 1], f32, tag="inv")
                nc.vector.reciprocal(inv, s)
                # hi = 127 - zp = 255 + round(mn * inv)
                hi = small.tile([P, 1], f32, tag="hi")
                nc.vector.tensor_tensor(hi, mn, inv, A.mult)
                nc.vector.tensor_scalar(hi, hi, MAGIC, None, A.add)
                nc.vector.tensor_scalar(hi, hi, MAGIC, 255.0, A.subtract, A.add)
                return s, inv, hi

            def quant_and_transpose(x_t, inv, hi, t):
                """n = min(round(x*inv), hi) as bf16, then DMA-xbar
                transpose (ACT ring) to nt[p, kk, tok]."""
                q = qpool.tile([P, I], f32, tag="q")
                n_bf = npool.tile([P, I], bf16, tag="n")
                nt = ntpool.tile([P, KK, P], bf16)
                nchunk = 4 if t == 0 else 1
                cw = I // nchunk
                ckk = KK // nchunk
                for c in range(nchunk):
                    sl = slice(c * cw, (c + 1) * cw)
                    nc.vector.tensor_scalar(q[:, sl], x_t[:, sl], inv, MAGIC,
                                            A.mult, A.add)
                    nc.vector.tensor_scalar(n_bf[:, sl], q[:, sl], MAGIC, hi,
                                            A.subtract, A.min)
                    nc.scalar.dma_start_transpose(
                        nt[:, c * ckk:(c + 1) * ckk, :], n_bf[:, sl])
                return nt

            stats = {0: quant_stats(x_tiles[0], 0)}
            nts = {0: quant_and_transpose(x_tiles[0], stats[0][1],
                                          stats[0][2], 0)}

            for tt in range(TT):
                if tt + 3 >= 3 and tt + 3 <= TT - 1:
                    x_t = xpool.tile([P, I], f32)
                    nc.gpsimd.dma_start(out=x_t, in_=xs[tt + 3][:, :])
                    x_tiles[tt + 3] = x_t
                if tt + 1 < TT:
                    stats[tt + 1] = quant_stats(x_tiles[tt + 1], tt + 1)
                    nts[tt + 1] = quant_and_transpose(
                        x_tiles[tt + 1], stats[tt + 1][1], stats[tt + 1][2],
                        tt + 1)
                    del x_tiles[tt + 1]

                nt = nts.pop(tt)
                s = stats.pop(tt)[0]
                ypsum = psum_y.tile([P, O], f32, tag="yp")
                y_sb = ypool.tile([P, O], f32)
                for kk in range(KK):
                    for j in range(NJ):
                        nc.tensor.matmul(
                            ypsum[:, j * NBANK:(j + 1) * NBANK],
                            lhsT=nt[:, kk, :],
                            rhs=wt_sb[:, kk, j * NBANK:(j + 1) * NBANK],
                            start=(kk == 0),
                            stop=(kk == KK - 1),
                        )
                # evict on DVE (copy * per-token scale), store on SP ring.
                nev = 4 if tt == TT - 1 else 2
                ew = O // nev
                for e in range(nev):
                    sl = slice(e * ew, (e + 1) * ew)
                    nc.vector.tensor_scalar_mul(y_sb[:, sl], ypsum[:, sl], s)
                    nc.sync.dma_start(out=ys[tt][:, sl], in_=y_sb[:, sl])

    _legalize_waits(nc)
    return nc


def kernel(x, w_q, w_scales, w_zeros):
    global _cached_nc, last_results
    if _cached_nc is None:
        _cached_nc = _build_nc()
    nc = _cached_nc

    x2 = np.ascontiguousarray(np.asarray(x, dtype=np.float32).reshape(TOK, I))
    s_e = np.repeat(np.asarray(w_scales, dtype=np.float32), GROUP, axis=1)
    z_e = np.repeat(np.asarray(w_zeros, dtype=np.float32), GROUP, axis=1)
    w_dq = (np.asarray(w_q).astype(np.float32) - z_e) * s_e
    wt = np.ascontiguousarray(w_dq.T).astype(ml_dtypes.bfloat16)

    in_maps = []
    for c in range(NCORES):
        m = {"wt": wt}
        for t in range(TT):
            base = c * TPC + t * P
            m[f"x{t:02d}"] = x2[base:base + P]
        in_maps.append(m)
    trace = os.environ.get("BASS_KERNEL_TRACE") == "1"
    res = run_bass_kernel_spmd(nc, in_maps, list(range(NCORES)), trace=trace)
    last_results = res
    out = np.concatenate(
        [res.results[c][f"y{t:02d}"] for c in range(NCORES) for t in range(TT)],
        axis=0,
    )
    return np.ascontiguousarray(out.reshape(B, S, O).astype(np.float32))
